# revision 1
# baseline (speedup 1.0000x reference)
"""CoOccurrenceLayer Trainium2 kernel (8 NeuronCores, data-parallel over batch).

Algorithm: out[p] = sum_{dq in 5x5} filt[dq] * co[idx[p], idx[p+dq]] * x[p+dq]
where idx = 16-bin quantization of exp(x) normalized by global min/max.

Key restructurings:
  * Binning is done with 15 precomputed x-space thresholds U_k (computed on
    host, bit-exact vs the fp32 reference chain): idx = sum_k (x >= U_k).
  * Images are laid out as 1D zero-padded streams (row stride 516) so all 25
    conv taps become pure free-dimension offsets on SBUF tiles.
  * Per-bin scatter V_j = x*(idx==j), 16 single-plane 5x5 convs C_j, 16x16
    co-mix D_i = sum_j co[i,j] C_j, final select out = sum_i (idx==i)*D_i.
"""

import sys

sys.path.insert(0, "/opt/trn_rl_repo")

import numpy as np

import concourse.bacc as bacc
import concourse.mybir as mybir
import concourse.tile as tile
from concourse import bass_utils
from concourse.ap import AP

# ---------------------------------------------------------------- constants
B, HH, WW = 64, 512, 512          # full input
NCORES = 8
BPC = B // NCORES                 # images per core
PW = WW + 4                       # padded row width (516)
PH = HH + 4
STREAM = PW * PH                  # 266256 padded stream length
LO = 2082                         # output positions per partition (128*LO >= STREAM)
HALO = 2 * PW + 2                 # 1034: max |tap offset|
LI = LO + 2 * HALO                # input slab length per partition
BUF = HALO + 128 * LO + HALO      # 268564 host stream buffer per image
NQ = 16
EPS = 1e-5

F16 = mybir.dt.float16
F32 = mybir.dt.float32
ALU = mybir.AluOpType


# ------------------------------------------------------- exact thresholds
def _g_chain(e, m, M):
    e = np.float32(e)
    norm = np.float32(np.float32(e - m) / M)
    t = np.float32(norm * np.float32(16.0))
    return np.float32(t - np.float32(EPS))


def _bin_of_e(e, m, M):
    return np.floor(np.abs(_g_chain(e, m, M))).astype(np.int32)


def _first_true(pred, x0):
    """Smallest fp32 v with pred(v) True; pred monotone nondecreasing."""
    lo = np.float32(x0)
    hi = np.float32(x0)
    step = np.float32(max(abs(float(x0)) * 3e-7, 1e-30))
    while pred(lo):
        lo = np.float32(lo - step)
        step = np.float32(step * 2)
    step = np.float32(max(abs(float(x0)) * 3e-7, 1e-30))
    while not pred(hi):
        hi = np.float32(hi + step)
        step = np.float32(step * 2)
    lo_b, hi_b = int(lo.view(np.int32)), int(hi.view(np.int32))
    while hi_b - lo_b > 1:
        mid_b = (lo_b + hi_b) // 2
        mid = np.int32(mid_b).view(np.float32)
        if pred(mid):
            hi_b = mid_b
        else:
            lo_b = mid_b
    return np.int32(hi_b).view(np.float32)


def compute_thresholds(x_min, x_max, exp_fn):
    """15 fp32 x-space boundaries: reference idx == sum_k (x >= U_k)."""
    m = exp_fn(np.float32(x_min))
    M = exp_fn(np.float32(x_max))
    U = np.zeros(15, np.float32)
    for k in range(1, NQ):
        e0 = np.float32(np.float64(m) + np.float64(M) * (k + EPS) / 16.0)
        T_k = _first_true(lambda e: _bin_of_e(e, m, M) >= k, e0)
        x0 = np.float32(np.log(max(np.float64(T_k), 1e-300)))
        U[k - 1] = _first_true(lambda v: exp_fn(v) >= T_k, x0)
    return U


# ------------------------------------------------------- device program
def build_program(U, co, filt):
    """U: [15] fp32 thresholds, co: [16,16] fp32, filt: [5,5] fp32."""
    nc = bacc.Bacc("TRN2", target_bir_lowering=False, debug=False)
    x_d = nc.dram_tensor("x", [BPC, BUF], F32, kind="ExternalInput").ap()
    o_d = nc.dram_tensor("out", [BPC, BUF], F32, kind="ExternalOutput").ap()

    taps = []  # (offset d, weight)
    for kh in range(5):
        for kw in range(5):
            taps.append(((kh - 2) * PW + (kw - 2), float(filt[kh, kw])))

    with tile.TileContext(nc) as tc:
        with (
            tc.tile_pool(name="xs", bufs=2) as p_xs,
            tc.tile_pool(name="f32t", bufs=1) as p_f32,
            tc.tile_pool(name="f16t", bufs=1) as p_f16,
            tc.tile_pool(name="cpl", bufs=1) as p_c,
            tc.tile_pool(name="sml", bufs=2) as p_sm,
            tc.tile_pool(name="ot", bufs=2) as p_o,
        ):
            for img in range(BPC):
                xs = p_xs.tile([128, LI], F32, tag="xs")
                src = AP(x_d.tensor, img * BUF, [[LO, 128], [1, LI]])
                nc.sync.dma_start(xs[:], src)

                idxf = p_f32.tile([128, LI], F32, tag="idxf")
                nc.vector.tensor_scalar(idxf[:], xs[:], float(U[0]), None, ALU.is_ge)
                for k in range(1, 15):
                    nc.vector.scalar_tensor_tensor(
                        idxf[:], xs[:], float(U[k]), idxf[:], ALU.is_ge, ALU.add
                    )
                idx16 = p_f16.tile([128, LI], F16, tag="idx16")
                nc.vector.tensor_copy(idx16[:], idxf[:])
                x16 = p_f16.tile([128, LI], F16, tag="x16")
                nc.vector.tensor_copy(x16[:], xs[:])

                cpl = p_c.tile([128, NQ * LO], F16, tag="cpl")
                vj = None
                for j in range(NQ):
                    vj = p_sm.tile([128, LI], F16, tag="vj")
                    nc.vector.scalar_tensor_tensor(
                        vj[:], idx16[:], float(j), x16[:], ALU.is_equal, ALU.mult
                    )
                    c_j = cpl[:, j * LO : (j + 1) * LO]
                    d0, w0 = taps[0]
                    nc.vector.tensor_scalar(
                        c_j, vj[:, HALO + d0 : HALO + d0 + LO], w0, None, ALU.mult
                    )
                    for d, w in taps[1:]:
                        nc.vector.scalar_tensor_tensor(
                            c_j, vj[:, HALO + d : HALO + d + LO], w, c_j,
                            ALU.mult, ALU.add,
                        )

                oacc = p_o.tile([128, LO], F16, tag="oacc")
                for i in range(NQ):
                    di = p_sm.tile([128, LO], F16, tag="di")
                    nc.vector.tensor_scalar(
                        di[:], cpl[:, 0:LO], float(co[i, 0]), None, ALU.mult
                    )
                    for j in range(1, NQ):
                        nc.vector.scalar_tensor_tensor(
                            di[:], cpl[:, j * LO : (j + 1) * LO], float(co[i, j]),
                            di[:], ALU.mult, ALU.add,
                        )
                    # select: oacc (+)= (idx==i) * di
                    mi = p_sm.tile([128, LO], F16, tag="mi")
                    nc.vector.scalar_tensor_tensor(
                        mi[:], idx16[:, HALO : HALO + LO], float(i), di[:],
                        ALU.is_equal, ALU.mult,
                    )
                    if i == 0:
                        nc.vector.tensor_copy(oacc[:], mi[:])
                    else:
                        nc.vector.tensor_tensor(oacc[:], oacc[:], mi[:], ALU.add)

                of32 = p_o.tile([128, LO], F32, tag="of32")
                nc.vector.tensor_copy(of32[:], oacc[:])
                dst = AP(o_d.tensor, img * BUF + HALO, [[LO, 128], [1, LO]])
                nc.sync.dma_start(dst, of32[:])

    nc.compile()
    return nc


# ------------------------------------------------------- host entry point
def kernel(x, co_matrix, spatial_filter):
    import jax.numpy as jnp

    x = np.asarray(x, np.float32)
    co = np.asarray(co_matrix, np.float32)
    filt = np.asarray(spatial_filter, np.float32)

    def exp_fn(v):
        return np.asarray(jnp.exp(np.float32(v)))

    U = compute_thresholds(x.min(), x.max(), exp_fn)

    # padded 1D streams
    imgs = x[:, 0]                                    # [64, 512, 512]
    padded = np.pad(imgs, ((0, 0), (2, 2), (2, 2)))   # [64, 516, 516]
    streams = np.zeros((B, BUF), np.float32)
    streams[:, HALO : HALO + STREAM] = padded.reshape(B, STREAM)

    nc = build_program(U, co, filt)
    in_maps = [{"x": streams[c * BPC : (c + 1) * BPC]} for c in range(NCORES)]
    res = bass_utils.run_bass_kernel_spmd(nc, in_maps, core_ids=list(range(NCORES)))

    out = np.empty((B, 1, HH, WW), np.float32)
    for c in range(NCORES):
        ob = res.results[c]["out"]                    # [BPC, BUF]
        o = ob[:, HALO : HALO + STREAM].reshape(BPC, PH, PW)
        out[c * BPC : (c + 1) * BPC, 0] = o[:, 2 : 2 + HH, 2 : 2 + WW]
    return out


# revision 6
# speedup vs baseline: 2.0095x; 2.0095x over previous
"""CoOccurrenceLayer Trainium2 kernel (8 NeuronCores, data-parallel over batch).

Algorithm: out[p] = sum_{dq in 5x5} filt[dq] * co[idx[p], idx[p+dq]] * x[p+dq]
where idx is a 16-bin quantization of exp(x) normalized by global min/max.

Structure:
  * Binning via 15 host-computed x-space thresholds (bit-exact vs the fp32
    reference chain): idx = sum_k (x >= U_k).
  * 2D-blocked layout: padded image [520, 528] in 8x16 blocks; partition =
    (r%8)*16 + c%16, free = block index. All 25 conv taps then live inside a
    2x2 neighborhood of input blocks.
  * Per-bin scatter V_j = x*(idx==j) on DVE; 5x5 conv of each V_j on the
    TensorEngine as 4 PSUM-accumulated matmuls against static
    filter-structured [128,128] weights; PSUM evacuated on ScalarE.
  * 16x16 co-mix + per-pixel select on DVE.
"""

import sys

sys.path.insert(0, "/opt/trn_rl_repo")

import numpy as np

import concourse.bacc as bacc
import concourse.mybir as mybir
import concourse.tile as tile
from concourse import bass_utils
from concourse.ap import AP

# ---------------------------------------------------------------- constants
B, HH, WW = 64, 512, 512
NCORES = 8
BPC = B // NCORES
NQ = 16
EPS = 1e-5

# V-grid (input, phase -2): padded rows r2=r+2 in [0,520), cols c2=c+2 in [0,528)
AV, BV = 65, 33            # V block grid (8x16 blocks)
NV = AV * BV               # 2145 V-blocks per image
AO, BO = 64, 32            # out block grid (phase 0, unpadded 512x512)
NO = AO * BO               # 2048 out-blocks per image

F16 = mybir.dt.float16
F32 = mybir.dt.float32
ALU = mybir.AluOpType


# ------------------------------------------------------- exact thresholds
def _g_chain(e, m, M):
    e = np.float32(e)
    norm = np.float32(np.float32(e - m) / M)
    t = np.float32(norm * np.float32(16.0))
    return np.float32(t - np.float32(EPS))


def _bin_of_e(e, m, M):
    return np.floor(np.abs(_g_chain(e, m, M))).astype(np.int32)


def _first_true(pred, x0):
    lo = np.float32(x0)
    hi = np.float32(x0)
    step = np.float32(max(abs(float(x0)) * 3e-7, 1e-30))
    while pred(lo):
        lo = np.float32(lo - step)
        step = np.float32(step * 2)
    step = np.float32(max(abs(float(x0)) * 3e-7, 1e-30))
    while not pred(hi):
        hi = np.float32(hi + step)
        step = np.float32(step * 2)
    lo_b, hi_b = int(lo.view(np.int32)), int(hi.view(np.int32))
    while hi_b - lo_b > 1:
        mid_b = (lo_b + hi_b) // 2
        mid = np.int32(mid_b).view(np.float32)
        if pred(mid):
            hi_b = mid_b
        else:
            lo_b = mid_b
    return np.int32(hi_b).view(np.float32)


def compute_thresholds(x_min, x_max, exp_fn):
    m = exp_fn(np.float32(x_min))
    M = exp_fn(np.float32(x_max))
    U = np.zeros(15, np.float32)
    for k in range(1, NQ):
        e0 = np.float32(np.float64(m) + np.float64(M) * (k + EPS) / 16.0)
        T_k = _first_true(lambda e: _bin_of_e(e, m, M) >= k, e0)
        x0 = np.float32(np.log(max(np.float64(T_k), 1e-300)))
        U[k - 1] = _first_true(lambda v: exp_fn(v) >= T_k, x0)
    return U


# ------------------------------------------------------- conv weights
def conv_lhst(filt):
    """4 static [128,128] f16 matrices; variant v=(da,db).
    out pixel (mr,mc) of out-block (a,b); k = (kr,kc) of V-block (a+da,b+db).
    lhsT[k, m] = F[dr+2, dc+2], dr = 8*da - 2 + kr - mr, dc = 16*db - 2 + kc - mc.
    """
    W = np.zeros((2, 2, 128, 128), np.float32)
    for da in range(2):
        for db in range(2):
            for kr in range(8):
                for kc in range(16):
                    for mr in range(8):
                        for mc in range(16):
                            dr = 8 * da - 2 + kr - mr
                            dc = 16 * db - 2 + kc - mc
                            if -2 <= dr <= 2 and -2 <= dc <= 2:
                                W[da, db, kr * 16 + kc, mr * 16 + mc] = filt[
                                    dr + 2, dc + 2
                                ]
    return W.reshape(4, 128, 128).astype(np.float16)


# ------------------------------------------------------- device program
def build_program(U, co, filt):
    nc = bacc.Bacc("TRN2", target_bir_lowering=False, debug=False)
    x_d = nc.dram_tensor("x", [BPC, 128, NV], F32, kind="ExternalInput").ap()
    w_d = nc.dram_tensor("wt", [128, 4 * 128], F16, kind="ExternalInput").ap()
    o_d = nc.dram_tensor("out", [BPC, 128, NO], F32, kind="ExternalOutput").ap()

    with tile.TileContext(nc) as tc:
        with (
            tc.tile_pool(name="wp", bufs=1) as p_w,
            tc.tile_pool(name="xs", bufs=2) as p_xs,
            tc.tile_pool(name="bins", bufs=1) as p_bin,
            tc.tile_pool(name="vpl", bufs=3) as p_v,
            tc.tile_pool(name="psum", bufs=2, space="PSUM") as p_ps,
            tc.tile_pool(name="cpl", bufs=1) as p_c,
            tc.tile_pool(name="sml", bufs=3) as p_sm,
            tc.tile_pool(name="ot", bufs=2) as p_o,
        ):
            wt = p_w.tile([128, 4 * 128], F16)
            nc.sync.dma_start(wt[:], w_d[:])

            for img in range(BPC):
                xs = p_xs.tile([128, NV], F32, tag="xs")
                nc.sync.dma_start(xs[:], x_d[img])

                idxf = p_bin.tile([128, NV], F32, tag="idxf")
                nc.vector.tensor_scalar(idxf[:], xs[:], float(U[0]), None, ALU.is_ge)
                for k in range(1, 15):
                    nc.vector.scalar_tensor_tensor(
                        idxf[:], xs[:], float(U[k]), idxf[:], ALU.is_ge, ALU.add
                    )
                idx16 = p_bin.tile([128, NV], F16, tag="idx16")
                nc.vector.tensor_copy(idx16[:], idxf[:])
                x16 = p_bin.tile([128, NV], F16, tag="x16")
                nc.vector.tensor_copy(x16[:], xs[:])

                # idx in out-grid layout (phase shift +2,+2): 4 region DMAs
                idxo = p_bin.tile([128, NO], F16, tag="idxo")
                src_t = idx16[:]
                dst_t = idxo[:]
                for rcase in range(2):
                    for ccase in range(2):
                        # out pixels mr in rows [0,6) or {6,7}; mc in [0,14) or {14,15}
                        nmr = 6 if rcase == 0 else 2
                        nmc = 14 if ccase == 0 else 2
                        mr0 = 0 if rcase == 0 else 6
                        mc0 = 0 if ccase == 0 else 14
                        # src free offset: +1 row-block / col-block when wrapped
                        soff = (1 if rcase else 0) * BV + (1 if ccase else 0)
                        for mr in range(mr0, mr0 + nmr):
                            spart = ((mr + 2) % 8) * 16 + ((mc0 + 2) % 16)
                            dpart = mr * 16 + mc0
                            src = AP(
                                src_t.tensor,
                                src_t.offset + spart * NV + soff,
                                [[NV, nmc], [BV, AO], [1, BO]],
                            )
                            dst = AP(
                                dst_t.tensor,
                                dst_t.offset + dpart * NO,
                                [[NO, nmc], [BO, AO], [1, BO]],
                            )
                            nc.sync.dma_start(dst, src)

                # per-bin scatter + PE conv
                cpl = p_c.tile([128, NQ * NO], F16, tag="cpl")
                for j in range(NQ):
                    vj = p_v.tile([128, NV], F16, tag="vj")
                    nc.vector.scalar_tensor_tensor(
                        vj[:], idx16[:], float(j), x16[:], ALU.is_equal, ALU.mult
                    )
                    ps = p_ps.tile([128, NO], F32, tag="ps")
                    vt = vj[:]
                    for ch in range(4):          # 512-col PSUM-bank chunks
                        for v in range(4):       # 2x2 neighbor patches
                            da, db = v >> 1, v & 1
                            rhs = AP(
                                vt.tensor,
                                vt.offset + (ch * 16 + da) * BV + db,
                                [[NV, 128], [BV, 16], [1, BO]],
                            )
                            nc.tensor.matmul(
                                ps[:, ch * 512 : (ch + 1) * 512],
                                wt[:, v * 128 : (v + 1) * 128],
                                rhs,
                                start=(v == 0),
                                stop=(v == 3),
                            )
                    nc.scalar.copy(cpl[:, j * NO : (j + 1) * NO], ps[:])

                # mix + select
                oacc = p_o.tile([128, NO], F16, tag="oacc")
                for i in range(NQ):
                    di = p_sm.tile([128, NO], F16, tag="di")
                    nc.vector.tensor_scalar(
                        di[:], cpl[:, 0:NO], float(co[i, 0]), None, ALU.mult
                    )
                    for j in range(1, NQ):
                        nc.vector.scalar_tensor_tensor(
                            di[:], cpl[:, j * NO : (j + 1) * NO], float(co[i, j]),
                            di[:], ALU.mult, ALU.add,
                        )
                    mi = p_sm.tile([128, NO], F16, tag="mi")
                    nc.vector.scalar_tensor_tensor(
                        mi[:], idxo[:], float(i), di[:], ALU.is_equal, ALU.mult
                    )
                    if i == 0:
                        nc.vector.tensor_copy(oacc[:], mi[:])
                    else:
                        nc.vector.tensor_tensor(oacc[:], oacc[:], mi[:], ALU.add)

                of32 = p_o.tile([128, NO], F32, tag="of32")
                nc.scalar.copy(of32[:], oacc[:])
                nc.sync.dma_start(o_d[img], of32[:])

    nc.compile()
    return nc


# ------------------------------------------------------- host packing
def pack_inputs(x):
    """x [64,1,512,512] f32 -> blocked [64, 128, NV] f32."""
    imgs = x[:, 0]
    xpad = np.pad(imgs, ((0, 0), (2, 6), (2, 14)))      # [64, 520, 528]
    xb = (
        xpad.reshape(B, AV, 8, BV, 16)
        .transpose(0, 2, 4, 1, 3)
        .reshape(B, 128, NV)
    )
    return np.ascontiguousarray(xb)


def unpack_outputs(res_list):
    out = np.empty((B, 1, HH, WW), np.float32)
    for c in range(NCORES):
        ob = res_list[c]["out"]                          # [BPC, 128, NO]
        o = (
            ob.reshape(BPC, 8, 16, AO, BO)
            .transpose(0, 3, 1, 4, 2)
            .reshape(BPC, HH, WW)
        )
        out[c * BPC : (c + 1) * BPC, 0] = o
    return out


def kernel(x, co_matrix, spatial_filter):
    import jax.numpy as jnp

    x = np.asarray(x, np.float32)
    co = np.asarray(co_matrix, np.float32)
    filt = np.asarray(spatial_filter, np.float32)

    def exp_fn(v):
        return np.asarray(jnp.exp(np.float32(v)))

    U = compute_thresholds(x.min(), x.max(), exp_fn)
    xb = pack_inputs(x)
    wts = np.ascontiguousarray(conv_lhst(filt).transpose(1, 0, 2).reshape(128, 4 * 128))

    nc = build_program(U, co, filt)
    in_maps = [
        {"x": xb[c * BPC : (c + 1) * BPC], "wt": wts} for c in range(NCORES)
    ]
    res = bass_utils.run_bass_kernel_spmd(nc, in_maps, core_ids=list(range(NCORES)))
    return unpack_outputs(res.results)


# revision 11
# speedup vs baseline: 6.4844x; 3.2268x over previous
"""CoOccurrenceLayer Trainium2 kernel (8 NeuronCores, data-parallel over batch).

Algorithm: out[p] = sum_{dq in 5x5} filt[dq] * co[idx[p], idx[p+dq]] * x[p+dq]
where idx is a 16-bin quantization of exp(x) normalized by global min/max.

Structure:
  * Binning via 15 host-computed x-space thresholds (bit-exact vs the fp32
    reference chain): idx = sum_k (x >= U_k).
  * 2D-blocked layout: padded image [520, 528] in 8x16 blocks; partition =
    (r%8)*16 + c%16, free = block index. All 25 conv taps live inside a 2x2
    neighborhood of input blocks.
  * Per-bin scatter V_j = x*(idx==j); 5x5 conv of each V_j on the
    TensorEngine as 4 PSUM-accumulated matmuls vs static filter-structured
    [128,128] weights.
  * Batched DMA-xbar transpose into channel-major layout; 16x16 co-mix and
    the 16->1 bin-select reduction also run on the TensorEngine.
"""

import sys

sys.path.insert(0, "/opt/trn_rl_repo")

import numpy as np

import concourse.bacc as bacc
import concourse.mybir as mybir
import concourse.tile as tile
from concourse import bass_utils
from concourse.ap import AP

# ---------------------------------------------------------------- constants
B, HH, WW = 64, 512, 512
NCORES = 8
BPC = B // NCORES
NQ = 16
EPS = 1e-5

AV, BV = 65, 33            # V-grid (phase -2) blocks of 8x16
NV = AV * BV               # 2145
AO, BO = 64, 32            # out grid (phase 0)
NO = AO * BO               # 2048 out-blocks
NG = NO // 8               # 256 block-groups of 8
GCH = 64                   # block-groups per strip chunk
NCH = NG // GCH            # 4 chunks
CW = GCH * 128             # 8192 strip columns per chunk

F16 = mybir.dt.float16
F32 = mybir.dt.float32
ALU = mybir.AluOpType


# ------------------------------------------------------- exact thresholds
def _g_chain(e, m, M):
    e = np.float32(e)
    norm = np.float32(np.float32(e - m) / M)
    t = np.float32(norm * np.float32(16.0))
    return np.float32(t - np.float32(EPS))


def _bin_of_e(e, m, M):
    return np.floor(np.abs(_g_chain(e, m, M))).astype(np.int32)


def _first_true(pred, x0):
    lo = np.float32(x0)
    hi = np.float32(x0)
    step = np.float32(max(abs(float(x0)) * 3e-7, 1e-30))
    while pred(lo):
        lo = np.float32(lo - step)
        step = np.float32(step * 2)
    step = np.float32(max(abs(float(x0)) * 3e-7, 1e-30))
    while not pred(hi):
        hi = np.float32(hi + step)
        step = np.float32(step * 2)
    lo_b, hi_b = int(lo.view(np.int32)), int(hi.view(np.int32))
    while hi_b - lo_b > 1:
        mid_b = (lo_b + hi_b) // 2
        mid = np.int32(mid_b).view(np.float32)
        if pred(mid):
            hi_b = mid_b
        else:
            lo_b = mid_b
    return np.int32(hi_b).view(np.float32)


def compute_thresholds(x_min, x_max, exp_fn):
    m = exp_fn(np.float32(x_min))
    M = exp_fn(np.float32(x_max))
    U = np.zeros(15, np.float32)
    for k in range(1, NQ):
        e0 = np.float32(np.float64(m) + np.float64(M) * (k + EPS) / 16.0)
        T_k = _first_true(lambda e: _bin_of_e(e, m, M) >= k, e0)
        x0 = np.float32(np.log(max(np.float64(T_k), 1e-300)))
        U[k - 1] = _first_true(lambda v: exp_fn(v) >= T_k, x0)
    return U


# ------------------------------------------------------- static PE weights
def build_weights(co, filt):
    """wt [128, 6*128] f16: 4 conv lhsT, 1 mix lhsT, 1 select-reduce lhsT."""
    W = np.zeros((6, 128, 128), np.float32)
    for da in range(2):
        for db in range(2):
            v = da * 2 + db
            for kr in range(8):
                for kc in range(16):
                    for mr in range(8):
                        for mc in range(16):
                            dr = 8 * da - 2 + kr - mr
                            dc = 16 * db - 2 + kc - mc
                            if -2 <= dr <= 2 and -2 <= dc <= 2:
                                W[v, kr * 16 + kc, mr * 16 + mc] = filt[dr + 2, dc + 2]
    # mix: k = (j, blk8), m = (i, blk8'): co[i, j] when blk8 == blk8'
    for j in range(16):
        for blk8 in range(8):
            for i in range(16):
                W[4, j * 8 + blk8, i * 8 + blk8] = co[i, j]
    # select-reduce: k = (i, blk8), m = blk8'
    for i in range(16):
        for blk8 in range(8):
            W[5, i * 8 + blk8, blk8] = 1.0
    return np.ascontiguousarray(
        W.astype(np.float16).transpose(1, 0, 2).reshape(128, 6 * 128)
    )


# ------------------------------------------------------- device program
def build_program(U, co, filt):
    nc = bacc.Bacc("TRN2", target_bir_lowering=False, debug=False)
    x_d = nc.dram_tensor("x", [BPC, 128, NV], F32, kind="ExternalInput").ap()
    w_d = nc.dram_tensor("wt", [128, 6 * 128], F16, kind="ExternalInput").ap()
    iv_d = nc.dram_tensor("ivec", [128, 1], F32, kind="ExternalInput").ap()
    o_d = nc.dram_tensor("out", [BPC, 8, NG * 128], F32, kind="ExternalOutput").ap()

    with tile.TileContext(nc) as tc:
        with (
            tc.tile_pool(name="wp", bufs=1) as p_w,
            tc.tile_pool(name="xs", bufs=2) as p_xs,
            tc.tile_pool(name="bins", bufs=1) as p_bin,
            tc.tile_pool(name="vpl", bufs=2) as p_v,
            tc.tile_pool(name="cps", bufs=1, space="PSUM") as p_cps,
            tc.tile_pool(name="dps", bufs=2, space="PSUM") as p_dps,
            tc.tile_pool(name="ops", bufs=2, space="PSUM") as p_ops,
            tc.tile_pool(name="cpl", bufs=1) as p_c,
            tc.tile_pool(name="str", bufs=1) as p_s,
            tc.tile_pool(name="ot", bufs=1) as p_o,
        ):
            wt = p_w.tile([128, 6 * 128], F16)
            nc.sync.dma_start(wt[:], w_d[:])
            ivec = p_w.tile([128, 1], F32)
            nc.sync.dma_start(ivec[:], iv_d[:])

            for img in range(BPC):
                xs = p_xs.tile([128, NV], F32, tag="xs")
                nc.sync.dma_start(xs[:], x_d[img])

                # --- binning (bit-exact): idx = sum_k (x >= U_k) ---
                idxf = p_bin.tile([128, NV], F32, tag="idxf")
                nc.vector.tensor_scalar(idxf[:], xs[:], float(U[0]), None, ALU.is_ge)
                for k in range(1, 15):
                    nc.vector.scalar_tensor_tensor(
                        idxf[:], xs[:], float(U[k]), idxf[:], ALU.is_ge, ALU.add
                    )
                idx16 = p_bin.tile([128, NV], F16, tag="idx16")
                nc.vector.tensor_copy(idx16[:], idxf[:])
                x16 = p_bin.tile([128, NV], F16, tag="x16")
                nc.scalar.copy(x16[:], xs[:])

                # --- idx in out-grid layout (phase shift +2,+2) ---
                idxo = p_bin.tile([128, NO], F16, tag="idxo")
                src_t = idx16[:]
                dst_t = idxo[:]
                for rcase in range(2):
                    for ccase in range(2):
                        nmr = 6 if rcase == 0 else 2
                        nmc = 14 if ccase == 0 else 2
                        mr0 = 0 if rcase == 0 else 6
                        mc0 = 0 if ccase == 0 else 14
                        soff = (1 if rcase else 0) * BV + (1 if ccase else 0)
                        for mr in range(mr0, mr0 + nmr):
                            spart = ((mr + 2) % 8) * 16 + ((mc0 + 2) % 16)
                            dpart = mr * 16 + mc0
                            src = AP(
                                src_t.tensor,
                                src_t.offset + spart * NV + soff,
                                [[NV, nmc], [BV, AO], [1, BO]],
                            )
                            dst = AP(
                                dst_t.tensor,
                                dst_t.offset + dpart * NO,
                                [[NO, nmc], [BO, AO], [1, BO]],
                            )
                            nc.sync.dma_start(dst, src)

                # --- per-bin scatter + PE conv; C stored j-interleaved:
                #     cpl[px, G*128 + j*8 + blk8] ---
                cpl = p_c.tile([128, NQ * NO], F16, tag="cpl")
                for j in range(NQ):
                    vj = p_v.tile([128, NV], F16, tag="vj")
                    nc.vector.scalar_tensor_tensor(
                        vj[:], idx16[:], float(j), x16[:], ALU.is_equal, ALU.mult
                    )
                    ps = p_cps.tile([128, NO], F32, tag="ps")
                    vt = vj[:]
                    for ch in range(4):
                        for v in range(4):
                            da, db = v >> 1, v & 1
                            rhs = AP(
                                vt.tensor,
                                vt.offset + (ch * 16 + da) * BV + db,
                                [[NV, 128], [BV, 16], [1, BO]],
                            )
                            nc.tensor.matmul(
                                ps[:, ch * 512 : (ch + 1) * 512],
                                wt[:, v * 128 : (v + 1) * 128],
                                rhs,
                                start=(v == 0),
                                stop=(v == 3),
                            )
                    cv = cpl[:]
                    dst = AP(
                        cv.tensor,
                        cv.offset + j * 8,
                        [cv.ap[0], [128, NG], [1, 8]],
                    )
                    if j % 2 == 0:
                        nc.scalar.copy(dst, ps[:])
                    else:
                        nc.vector.tensor_copy(dst, ps[:])

                # --- strip chunks: transpose -> mix -> select -> reduce ---
                for sc in range(NCH):
                    g0 = sc * GCH
                    # C chunk to channel-major: ct[(j*8+blk8), g*128+px]
                    ct = p_s.tile([128, GCH, 128], F16, tag="ct")
                    nc.sync.dma_start_transpose(
                        ct[:], cpl[:, g0 * 128 : (g0 + GCH) * 128]
                    )
                    # replicate idxo 16x in the j-slot, then transpose
                    rep = p_s.tile([128, GCH, 128], F16, tag="rep")
                    rv = rep[:]
                    for i in range(NQ):
                        dst = AP(
                            rv.tensor,
                            rv.offset + i * 8,
                            [rv.ap[0], [128, GCH], [1, 8]],
                        )
                        if i % 2 == 0:
                            nc.vector.tensor_copy(dst, idxo[:, g0 * 8 : g0 * 8 + GCH * 8])
                        else:
                            nc.scalar.copy(dst, idxo[:, g0 * 8 : g0 * 8 + GCH * 8])
                    idxt = p_s.tile([128, GCH, 128], F16, tag="idxt")
                    nc.sync.dma_start_transpose(idxt[:], rep[:])
                    # mask = (idx_rep == i(partition))
                    mask = rep[:].rearrange("p a b -> p (a b)")
                    nc.vector.tensor_scalar(
                        mask, idxt[:].rearrange("p a b -> p (a b)"), ivec[:],
                        None, ALU.is_equal,
                    )
                    # mix on PE: D[(i,blk8), n] = sum_j co[i,j] C[(j,blk8), n]
                    dt = p_s.tile([128, CW], F16, tag="dt")
                    ctf = ct[:].rearrange("p a b -> p (a b)")
                    for sub in range(CW // 512):
                        psd = p_dps.tile([128, 512], F32, tag="psd")
                        nc.tensor.matmul(
                            psd[:],
                            wt[:, 4 * 128 : 5 * 128],
                            ctf[:, sub * 512 : (sub + 1) * 512],
                            start=True,
                            stop=True,
                        )
                        if sub % 2 == 0:
                            nc.scalar.copy(dt[:, sub * 512 : (sub + 1) * 512], psd[:])
                        else:
                            nc.vector.tensor_copy(
                                dt[:, sub * 512 : (sub + 1) * 512], psd[:]
                            )
                    # select: mask *= D, then reduce over the 16 i-groups on PE
                    nc.vector.tensor_tensor(
                        mask[:, : CW // 2], mask[:, : CW // 2], dt[:, : CW // 2],
                        ALU.mult,
                    )
                    nc.gpsimd.tensor_tensor(
                        mask[:, CW // 2 :], mask[:, CW // 2 :], dt[:, CW // 2 :],
                        ALU.mult,
                    )
                    osb = p_o.tile([8, CW], F32, tag="osb")
                    for sub in range(CW // 512):
                        pso = p_ops.tile([8, 512], F32, tag="pso")
                        nc.tensor.matmul(
                            pso[:],
                            wt[:, 5 * 128 : 5 * 128 + 8],
                            mask[:, sub * 512 : (sub + 1) * 512],
                            start=True,
                            stop=True,
                        )
                        if sub % 2 == 0:
                            nc.vector.tensor_copy(
                                osb[:, sub * 512 : (sub + 1) * 512], pso[:]
                            )
                        else:
                            nc.scalar.copy(osb[:, sub * 512 : (sub + 1) * 512], pso[:])
                    nc.sync.dma_start(
                        o_d[img, :, g0 * 128 : (g0 + GCH) * 128], osb[:]
                    )

    nc.compile()
    return nc


# ------------------------------------------------------- host packing
def pack_inputs(x):
    imgs = x[:, 0]
    xpad = np.pad(imgs, ((0, 0), (2, 6), (2, 14)))      # [64, 520, 528]
    xb = (
        xpad.reshape(B, AV, 8, BV, 16)
        .transpose(0, 2, 4, 1, 3)
        .reshape(B, 128, NV)
    )
    return np.ascontiguousarray(xb)


def unpack_outputs(res_list):
    out = np.empty((B, 1, HH, WW), np.float32)
    for c in range(NCORES):
        ob = res_list[c]["out"]                          # [BPC, 8, NG*128]
        # ob[img, blk8, G, px]: n_o = G*8+blk8 = a*32+b; px = mr*16+mc
        o = ob.reshape(BPC, 8, 64, 4, 8, 16)             # blk8, a, bhi, mr, mc
        o = o.transpose(0, 2, 4, 3, 1, 5)                # img, a, mr, bhi, blk8, mc
        out[c * BPC : (c + 1) * BPC, 0] = o.reshape(BPC, HH, WW)
    return out


def make_ivec():
    iv = (np.arange(128) // 8).astype(np.float32).reshape(128, 1)
    return np.ascontiguousarray(iv)


def kernel(x, co_matrix, spatial_filter):
    import jax.numpy as jnp

    x = np.asarray(x, np.float32)
    co = np.asarray(co_matrix, np.float32)
    filt = np.asarray(spatial_filter, np.float32)

    def exp_fn(v):
        return np.asarray(jnp.exp(np.float32(v)))

    U = compute_thresholds(x.min(), x.max(), exp_fn)
    xb = pack_inputs(x)
    wts = build_weights(co, filt)
    iv = make_ivec()

    nc = build_program(U, co, filt)
    in_maps = [
        {"x": xb[c * BPC : (c + 1) * BPC], "wt": wts, "ivec": iv}
        for c in range(NCORES)
    ]
    res = bass_utils.run_bass_kernel_spmd(nc, in_maps, core_ids=list(range(NCORES)))
    return unpack_outputs(res.results)
